# revision 16
# baseline (speedup 1.0000x reference)
"""Chamfer distance on 8 TRN2 NeuronCores.

Problem: x [4, 3, 4096], y [4, 3, 4096] f32.
  dist[b, n, m] = sum_d (x[b,d,n] - y[b,d,m])^2
  out = mean_b( sum_n min_m dist + sum_m min_n dist )

Strategy (v2.5):
  - Shard: core c handles batch b = c//2, n-half h = c%2 (2048 rows x 4096 cols
    of the distance matrix per core).
  - dist = |x|^2 + |y|^2 - 2 x.y as a single K=24 bf16 matmul per output tile
    (fp32 operands Dekker-split into bf16 triples on the host; products carry
    ~fp32 precision).  PSUM accumulates fp32 in [128, 2048] strips.
  - ScalarE evacuates each strip to fp16 (cp tile [128, 2, 2048]).  Engine
    balance: PE ~60us (1.2 GHz effective), ACT ~62us, DVE ~70us per core.
  - VectorE per strip, right after its evacuation (fp16 2x_1p mode):
      col pass: colacc[:, g, :] = min(colacc[:, g, :], cp[:, g, :])
      row pass: L1 halving in place (tile 0 into a scratch so its raw cp
      survives for tile 1's colacc init, which fuses away the init copy).
  - Per tile: L2 into a half-specific gather buffer; after tiles 7/15 a
    batched halving tree reduces that half's gather buffer to rmin slices
    (letting the scheduler overlap the first half's tree with later tiles).
  - Col fold: PE-transpose 128-col chunks of colacc into PSUM fp16
    [128, 16, 128] tiles; 2 tensor_reduce -> cmin [128, 32].
  - Host: rmin strip-pair mins + sum; cmin min-combined across the two
    half-shard cores per batch, summed; mean over batches.

build_nc(loop_n) wraps the whole pipeline in a hardware For_i loop executing
it loop_n times (used by test.py for slope-based HW timing; the graded kernel
uses loop_n=1 with no loop).
"""

import numpy as np
import ml_dtypes
from contextlib import ExitStack

import concourse.bass as bass
import concourse.mybir as mybir
import concourse.tile as tile
from concourse import bacc
from concourse.bass import ts, ds
from concourse.bass_utils import run_bass_kernel_spmd

B, D, N, M = 4, 3, 4096, 4096
NCORES = 8
HALF = N // 2            # rows of the distance matrix per core
NT = HALF // 128         # 16 row tiles per core
KROWS = 24               # contraction rows of the lifted matmul

bf16 = ml_dtypes.bfloat16

# stash of the last BassKernelResults (test.py reads this)
last_results = None
_NC_CACHE = {}


def build_nc(loop_n: int = 1) -> bass.Bass:
    nc = bacc.Bacc()
    f32 = mybir.dt.float32
    f16 = mybir.dt.float16
    bft = mybir.dt.bfloat16
    mn = mybir.AluOpType.min

    lhsT_d = nc.declare_dram_parameter("lhsT", [KROWS, HALF], bft, isOutput=False)
    rhs_d = nc.declare_dram_parameter("rhs", [KROWS, M], bft, isOutput=False)
    ident_d = nc.declare_dram_parameter("ident", [128, 128], f16, isOutput=False)
    rmin_d = nc.declare_dram_parameter("rmin", [128, NT, 2], f16, isOutput=True)
    cmin_d = nc.declare_dram_parameter("cmin", [128, 32], f16, isOutput=True)

    with tile.TileContext(nc) as tc, ExitStack() as ctx:
        consts = ctx.enter_context(tc.tile_pool(name="consts", bufs=1))
        cp_pool = ctx.enter_context(tc.tile_pool(name="cp", bufs=4))
        ps_pool = ctx.enter_context(tc.tile_pool(name="ps", bufs=2, space="PSUM"))

        lhsT_sb = consts.tile([KROWS, HALF], bft)
        rhs_sb = consts.tile([KROWS, M], bft)
        nc.sync.dma_start(out=lhsT_sb[:, :], in_=lhsT_d[:, :])
        # rhs in two halves so the first tile's matmuls start sooner
        nc.sync.dma_start(out=rhs_sb[:, 0:2048], in_=rhs_d[:, 0:2048])
        nc.sync.dma_start(out=rhs_sb[:, 2048:M], in_=rhs_d[:, 2048:M])

        ident = consts.tile([128, 128], f16)
        nc.sync.dma_start(out=ident[:, :], in_=ident_d[:, :])

        colacc = consts.tile([128, 2, 2048], f16)
        l1scr = consts.tile([128, 2, 1024], f16)
        gbufA = consts.tile([128, NT // 2, 2, 512], f16)
        gbufB = consts.tile([128, NT // 2 - 1, 2, 512], f16)  # tiles 8..14
        rmin_sb = consts.tile([128, NT, 2], f16)
        cmin_sb = consts.tile([128, 32], f16)

        def body(_i=None):
            cp0 = None
            for t in range(NT):
                cp = cp_pool.tile([128, 2, 2048], f16, tag="cp")
                if t == 0:
                    cp0 = cp
                l1 = l1scr if t == 0 else cp
                for g in range(2):
                    pd = ps_pool.tile([128, 2048], f32, tag="pd")
                    for j in range(4):
                        nc.tensor.matmul(
                            pd[:, ts(j, 512)],
                            lhsT_sb[:, ts(t, 128)],
                            rhs_sb[:, ds(g * 2048 + j * 512, 512)],
                            start=True,
                            stop=True,
                        )
                    # ScalarE evacuates the strip to fp16
                    nc.scalar.copy(cp[:, g, :], pd[:, :])
                    # col pass per strip (tile 0 defers; tile 1 merges cp0,
                    # which fuses away a separate colacc init copy)
                    if t == 1:
                        nc.vector.tensor_tensor(
                            out=colacc[:, g, :], in0=cp0[:, g, :],
                            in1=cp[:, g, :], op=mn,
                        )
                    elif t > 1:
                        nc.vector.tensor_tensor(
                            out=colacc[:, g, :], in0=colacc[:, g, :],
                            in1=cp[:, g, :], op=mn,
                        )
                    # row pass L1 per strip (pairs columns j, j+1024 within
                    # the strip; tile 0 writes a scratch to keep cp raw)
                    nc.vector.tensor_tensor(
                        out=l1[:, g, 0:1024], in0=cp[:, g, 0:1024],
                        in1=cp[:, g, 1024:2048], op=mn,
                    )
                if t == NT - 1:
                    # the last tile keeps a private deep tree so the batched
                    # gbufB epilogue (tiles 8..14) can run during this tile's
                    # matmuls/evacuation -- shortens the serial tail
                    w = 1024
                    while w > 64:
                        nc.vector.tensor_tensor(
                            out=cp[:, :, 0 : w // 2],
                            in0=cp[:, :, 0 : w // 2],
                            in1=cp[:, :, w // 2 : w],
                            op=mn,
                        )
                        w //= 2
                    nc.vector.tensor_reduce(
                        out=rmin_sb[:, NT - 1, :],
                        in_=cp[:, :, 0:64],
                        axis=mybir.AxisListType.X,
                        op=mn,
                    )
                else:
                    # L2 into this half's gather slot
                    gb, sl = (gbufA, t) if t < NT // 2 else (gbufB, t - NT // 2)
                    nc.vector.tensor_tensor(
                        out=gb[:, sl, :, :], in0=l1[:, :, 0:512],
                        in1=l1[:, :, 512:1024], op=mn,
                    )
                if t == NT // 2 - 1 or t == NT - 2:
                    # batched halving tree for the finished half:
                    # [128, 8|7, 2, 512] -> [..., 16] -> rmin slice
                    gb = gbufA if t == NT // 2 - 1 else gbufB
                    lo = 0 if t == NT // 2 - 1 else NT // 2
                    nslots = NT // 2 if t == NT // 2 - 1 else NT // 2 - 1
                    w = 512
                    while w > 16:
                        nc.vector.tensor_tensor(
                            out=gb[:, :, :, 0 : w // 2],
                            in0=gb[:, :, :, 0 : w // 2],
                            in1=gb[:, :, :, w // 2 : w],
                            op=mn,
                        )
                        w //= 2
                    nc.vector.tensor_reduce(
                        out=rmin_sb[:, lo : lo + nslots, :],
                        in_=gb[:, :, :, 0:16],
                        axis=mybir.AxisListType.X,
                        op=mn,
                    )
            nc.sync.dma_start(out=rmin_d[:, :, :], in_=rmin_sb[:, :, :])

            # col fold: PE-transpose 128-col chunks, then batched min-reduce
            for grp in range(2):
                pt = ps_pool.tile([128, 16, 128], f16, tag="pd")
                for i in range(16):
                    k = grp * 16 + i
                    g, jc = divmod(k, 16)
                    nc.tensor.transpose(
                        pt[:, i, :], colacc[:, g, ts(jc, 128)], ident
                    )
                nc.vector.tensor_reduce(
                    out=cmin_sb[:, ts(grp, 16)],
                    in_=pt[:, :, :],
                    axis=mybir.AxisListType.X,
                    op=mn,
                )
            nc.sync.dma_start(out=cmin_d[:, :], in_=cmin_sb[:, :])

        if loop_n == 1:
            body()
        else:
            with tc.For_i(0, loop_n, 1) as i:
                body(i)

    nc.compile()
    return nc


def _get_nc(loop_n: int = 1) -> bass.Bass:
    if loop_n not in _NC_CACHE:
        _NC_CACHE[loop_n] = build_nc(loop_n)
    return _NC_CACHE[loop_n]


def _split3(v: np.ndarray):
    """Split float64 array into three bf16 terms summing to v (err ~2^-27|v|)."""
    a = v.astype(bf16)
    r = v - a.astype(np.float64)
    b = r.astype(bf16)
    r2 = r - b.astype(np.float64)
    c = r2.astype(bf16)
    return a, b, c


def build_operands(xs: np.ndarray, ys: np.ndarray):
    """Lift one core's shard into the K=24 bf16 matmul operands.

    xs: [3, HALF] f32 (x coords of this core's rows)
    ys: [3, M] f32 (full y for this batch)
    Returns lhsT [24, HALF] bf16, rhs [24, M] bf16 with
      (lhsT.T @ rhs)[n, m] ~= |x_n|^2 + |y_m|^2 - 2 x_n . y_m
    """
    xs64 = xs.astype(np.float64)
    ys64 = ys.astype(np.float64)
    u = -2.0 * xs64
    xsq = (xs64 * xs64).sum(axis=0)
    ysq = (ys64 * ys64).sum(axis=0)

    uh, um, ul = _split3(u)      # [3, HALF] each
    vh, vm, vl = _split3(ys64)   # [3, M] each
    xqh, xqm, xql = _split3(xsq)
    yqh, yqm, yql = _split3(ysq)
    ones_l = np.ones(HALF, dtype=bf16)
    ones_m = np.ones(M, dtype=bf16)

    lhs_rows, rhs_rows = [], []
    for d in range(D):
        for a, b_ in ((uh, vh), (uh, vm), (uh, vl), (um, vh), (um, vm), (ul, vh)):
            lhs_rows.append(a[d])
            rhs_rows.append(b_[d])
    for yq in (yqh, yqm, yql):
        lhs_rows.append(ones_l)
        rhs_rows.append(yq)
    for xq in (xqh, xqm, xql):
        lhs_rows.append(xq)
        rhs_rows.append(ones_m)

    lhsT = np.ascontiguousarray(np.stack(lhs_rows))
    rhs = np.ascontiguousarray(np.stack(rhs_rows))
    assert lhsT.shape == (KROWS, HALF) and rhs.shape == (KROWS, M)
    return lhsT, rhs


_IDENT = np.eye(128, dtype=np.float16)


def make_in_maps(x: np.ndarray, y: np.ndarray):
    in_maps = []
    for c in range(NCORES):
        b, h = divmod(c, 2)
        lhsT, rhs = build_operands(x[b][:, h * HALF : (h + 1) * HALF], y[b])
        in_maps.append({"lhsT": lhsT, "rhs": rhs, "ident": _IDENT})
    return in_maps


def combine_results(results):
    totals = []
    for b in range(B):
        r0 = results[2 * b]
        r1 = results[2 * b + 1]
        xsum = 0.0
        for r in (r0, r1):
            rm = np.asarray(r["rmin"], np.float64)   # [128, NT, 2]
            xsum += rm.min(axis=2).sum()
        cm = np.minimum(
            np.asarray(r0["cmin"], np.float64), np.asarray(r1["cmin"], np.float64)
        )
        totals.append(xsum + cm.sum())
    return np.float32(np.mean(totals))


def kernel(x: np.ndarray, y: np.ndarray) -> np.ndarray:
    global last_results
    x = np.asarray(x, dtype=np.float32)
    y = np.asarray(y, dtype=np.float32)
    assert x.shape == (B, D, N) and y.shape == (B, D, M)
    in_maps = make_in_maps(x, y)
    res = run_bass_kernel_spmd(_get_nc(), in_maps, list(range(NCORES)))
    last_results = res
    return combine_results(res.results)
